# revision 13
# baseline (speedup 1.0000x reference)
"""Trainium2 Bass kernel for the GPCwSTU rollout (nn_GPCwSTU_72576357368005).

Math restructure (v2): the sequential rollout is the lower-triangular system
    u_t = d_t - K x_t,   x_t = Ecat^T p_t,   p_t = sum_{s<t} phi_s (x) u_s
with d_t = bias + sum_i E[:,:,i] @ w_{t-4+i} precomputable in parallel.
Richardson iteration in COMPOSED form (no Fmat precompute):
    u <- d - K @ scan_t(Ecat^T (phi (x) u))
Each iteration materializes X = scan(Ecat^T O) as a side product; after k
iterations, u = u^k and X = X(u^{k-1}) (one-iteration-stale X; fp64+bf16
mirror gives loss rel-err 1.7e-3 at k=2 vs the 2e-2 gate).

Time is sharded 256 steps/core across 8 cores; per iteration one AllGather of
per-core block sums of the z-steps provides the cross-core prefix offsets.
The scan runs offset-free (init 0) so the collective overlaps scan+stage2;
offsets are then folded in as per-partition scalars: u -= K@offx, X += offx.

Layouts are feature-major ([feature, t]). Phase 1 runs in float32r (1 PE
cycle/row at moving-dim 256 vs 4 for fp32).
"""

import os
import sys

sys.path.insert(0, "/opt/trn_rl_repo")

import numpy as np
import ml_dtypes

import concourse.bass as bass
import concourse.bacc as bacc
import concourse.mybir as mybir
from concourse import tile
from concourse.bass_utils import run_bass_kernel_spmd

BF16 = mybir.dt.bfloat16
F32 = mybir.dt.float32
F32R = mybir.dt.float32r
AL = mybir.AluOpType

T, N, MC, KF, M = 2048, 1024, 512, 20, 5
NCORES = 8
TS = T // NCORES          # 256 timesteps per core
NK = N // 128             # 8 tiles over state dim
CT = MC // 128            # 4 tiles over control dim
ICT = (KF * MC) // 128    # 80 tiles over the (filter, control) axis
NITERS = int(os.environ.get("K_NITERS", "2"))
SKIP_COLL = bool(int(os.environ.get("K_SKIP_COLL", "0")))  # timing expt only
GRP = 4                   # Ecat chunks per DMA group (1 MB transfers)
NGRP = ICT // GRP
BRES = int(os.environ.get("K_BRES", "12"))  # groups kept resident in SBUF
OBUF = 12                 # rotating O-chunk buffers

P1DT = F32R               # phase-1 matmul dtype (float32r: 4x faster than f32)

_CACHE = {}


def build_nc(debug=False, reps=1):
    nc = bacc.Bacc(None, target_bir_lowering=False, debug=False)

    # ---- I/O ----
    wT_d = nc.declare_dram_parameter("wT", [N, TS + M - 1], P1DT, isOutput=False)
    ET_d = nc.declare_dram_parameter("ET", [M, N, MC], P1DT, isOutput=False)
    Ecat_d = nc.declare_dram_parameter("Ecat", [KF * MC, N], BF16, isOutput=False)
    KT_d = nc.declare_dram_parameter("KT", [N, MC], BF16, isOutput=False)
    Q_d = nc.declare_dram_parameter("Q", [N, N], BF16, isOutput=False)
    R_d = nc.declare_dram_parameter("R", [MC, MC], BF16, isOutput=False)
    phiB_d = nc.declare_dram_parameter("phiB", [128, KF, TS], BF16, isOutput=False)
    biasT_d = nc.declare_dram_parameter("biasT", [MC, 1], F32, isOutput=False)
    mask_d = nc.declare_dram_parameter("mask", [NCORES, 1], F32, isOutput=False)
    loss_d = nc.declare_dram_parameter("loss", [1, TS], F32, isOutput=True)
    if debug:
        dbg_d = nc.declare_dram_parameter("dbg_d", [128, CT, TS], F32, isOutput=True)
        dbg_u1 = nc.declare_dram_parameter("dbg_u1", [128, CT, TS], F32, isOutput=True)
        dbg_uf = nc.declare_dram_parameter("dbg_uf", [128, CT, TS], F32, isOutput=True)
        dbg_X = nc.declare_dram_parameter("dbg_X", [128, NK, TS], F32, isOutput=True)

    # collective bounce buffers
    bxsum_d = nc.dram_tensor("bxsum", [N], F32)
    bxgat_d = nc.dram_tensor("bxgat", [NCORES, N], F32, addr_space="Shared")

    with tile.TileContext(nc) as tc:
        with (
            tc.tile_pool(name="const", bufs=1) as cpool,
            tc.tile_pool(name="live", bufs=1) as opool,
            tc.tile_pool(name="work", bufs=2) as wpool,
        ):
            # ---- small constants ----
            KTs = cpool.tile([128, NK, MC], BF16)
            nc.scalar.dma_start(KTs[:], KT_d.ap().rearrange("(k p) c -> p k c", p=128))
            phiB = cpool.tile([128, KF, TS], BF16)
            nc.scalar.dma_start(phiB[:], phiB_d[:])
            biasT = cpool.tile([128, CT, 1], F32)
            nc.scalar.dma_start(biasT[:], biasT_d.ap().rearrange("(c p) one -> p c one", p=128))
            mask = cpool.tile([NCORES, 1], F32)
            nc.scalar.dma_start(mask[:], mask_d[:])
            zeros = cpool.tile([128, TS], F32)
            nc.vector.memset(zeros[:], 0.0)
            ones = cpool.tile([128, 1], BF16)
            nc.vector.memset(ones[:], 1.0)
            gat0 = None
            if SKIP_COLL:
                gat0 = cpool.tile([NCORES, N], F32)
                nc.vector.memset(gat0[:], 0.0)

            for rep in range(reps):
                # long-lived per-rep state
                d = opool.tile([128, CT, TS], F32)
                ubf = opool.tile([128, CT, TS], BF16)
                Xbf = opool.tile([128, NK, TS], BF16)

                with tc.tile_pool(name="ps", bufs=1, space="PSUM") as pspool:
                    # single PSUM tile: plane k <-> bank k
                    zps = pspool.tile([128, NK, 512], F32)

                    # ---- phase 1: d = bias + sum_i E_i @ w_shift_i ----
                    with tc.tile_pool(name="p1", bufs=2) as p1:
                        wTs = p1.tile([128, NK, TS + M - 1], P1DT, tag="wts")
                        nc.sync.dma_start(
                            wTs[:], wT_d.ap().rearrange("(k p) t -> p k t", p=128))
                        ETr = ET_d.ap().rearrange("i (k p) c -> p i k c", p=128)
                        for i in range(M):
                            ETs = p1.tile([128, NK, MC], P1DT, tag="ets")
                            eng = nc.scalar if i % 2 == 0 else nc.sync
                            eng.dma_start(ETs[:], ETr[:, i])
                            for ct in range(CT):
                                for k in range(NK):
                                    nc.tensor.matmul(
                                        zps[:, ct, 0:TS],
                                        ETs[:, k, ct * 128:(ct + 1) * 128],
                                        wTs[:, k, i:i + TS],
                                        start=(i == 0 and k == 0),
                                        stop=(i == M - 1 and k == NK - 1),
                                    )
                        for ct in range(CT):
                            nc.vector.tensor_scalar_add(
                                d[:, ct, :], zps[:, ct, 0:TS], biasT[:, ct, :])
                            nc.vector.tensor_copy(ubf[:, ct, :], d[:, ct, :])
                        if debug and rep == 0:
                            nc.sync.dma_start(dbg_d[:], d[:])

                    # Q/R loads overlap the iterations; needed only in phase 5
                    p5c = tc.alloc_tile_pool(name="p5c", bufs=1)
                    Qs = p5c.tile([128, NK, N], BF16)
                    nc.gpsimd.dma_start(Qs[:], Q_d.ap().rearrange("(k p) n -> p k n", p=128))
                    Rs = p5c.tile([128, CT, MC], BF16)
                    nc.gpsimd.dma_start(Rs[:], R_d.ap().rearrange("(k p) c -> p k c", p=128))

                    # ---- Richardson iterations (composed form) ----
                    with (
                        tc.tile_pool(name="eres", bufs=1) as erpool,
                        tc.tile_pool(name="ecat", bufs=3) as epool,
                    ):
                        # first BRES groups stay resident after iteration 1
                        eres = erpool.tile([128, BRES * GRP, N], BF16)
                        for it in range(NITERS):
                            last = it == NITERS - 1
                            # stage 1: zsteps = Ecat^T @ (phi (x) u), with the
                            # O chunks built just-in-time from a rotating pool
                            for g in range(NGRP):
                                if g < BRES:
                                    eg = eres[:, g * GRP:(g + 1) * GRP, :]
                                    if it == 0:
                                        eng = nc.sync if g % 2 == 0 else nc.scalar
                                        eng.dma_start(
                                            eg,
                                            Ecat_d[g * GRP * 128:(g + 1) * GRP * 128, :]
                                            .rearrange("(a p) n -> p a n", p=128))
                                else:
                                    eg = epool.tile([128, GRP, N], BF16, tag="eg")
                                    eng = nc.sync if g % 2 == 0 else nc.scalar
                                    eng.dma_start(
                                        eg[:],
                                        Ecat_d[g * GRP * 128:(g + 1) * GRP * 128, :]
                                        .rearrange("(a p) n -> p a n", p=128))
                                for a in range(GRP):
                                    kk = g * GRP + a
                                    Oc = opool.tile([128, TS], BF16, tag="oc",
                                                    bufs=OBUF, name="Oc")
                                    nc.vector.tensor_tensor(
                                        Oc[:], ubf[:, kk % CT, :],
                                        phiB[:, kk // CT, :], op=AL.mult)
                                    for nt in range(NK):
                                        nc.tensor.matmul(
                                            zps[:, nt, 0:TS],
                                            eg[:, a, nt * 128:(nt + 1) * 128],
                                            Oc[:],
                                            start=(kk == 0), stop=(kk == ICT - 1),
                                        )
                            # block sums -> AllGather (overlaps scan+stage2)
                            Bz = wpool.tile([128, NK, 1], F32, tag="bz")
                            for nt in range(NK):
                                nc.vector.reduce_sum(Bz[:, nt, :], zps[:, nt, 0:TS],
                                                     axis=mybir.AxisListType.X)
                                nc.gpsimd.dma_start(
                                    bxsum_d[nt * 128:(nt + 1) * 128], Bz[:, nt, :])
                            if not SKIP_COLL:
                                nc.gpsimd.collective_compute(
                                    "AllGather", AL.bypass,
                                    ins=[bxsum_d[:]], outs=[bxgat_d[:]],
                                    replica_groups=[list(range(NCORES))],
                                )
                                gatx = wpool.tile([NCORES, N], F32, tag="gatx")
                                nc.gpsimd.dma_start(gatx[:], bxgat_d[:])
                            else:
                                gatx = gat0
                            # X = exclusive-scan(zsteps), offset-free
                            for nt in range(NK):
                                nc.vector.tensor_copy(Xbf[:, nt, 0:1], zeros[:, 0:1])
                                nc.vector.tensor_tensor_scan(
                                    Xbf[:, nt, 1:TS], zps[:, nt, 0:TS - 1],
                                    zeros[:, 0:TS - 1], 0.0,
                                    op0=AL.add, op1=AL.add)
                            # stage 2: uL = d - K @ Xlocal
                            for ct in range(CT):
                                for k in range(NK):
                                    nc.tensor.matmul(
                                        zps[:, 4 + ct, 256:512],
                                        KTs[:, k, ct * 128:(ct + 1) * 128],
                                        Xbf[:, k, :],
                                        start=(k == 0), stop=(k == NK - 1),
                                    )
                            for ct in range(CT):
                                nc.vector.tensor_sub(
                                    ubf[:, ct, :], d[:, ct, :],
                                    zps[:, 4 + ct, 256:512])
                            # offsets: offx = sum_{r'<r} Bz_{r'};  c = K @ offx
                            offS = wpool.tile([128, NK, 1], F32, tag="offs")
                            offB = wpool.tile([128, NK, 1], BF16, tag="offb")
                            for nt in range(NK):
                                nc.tensor.matmul(
                                    zps[:, nt, 504:505],
                                    gatx[:, nt * 128:(nt + 1) * 128], mask[:],
                                    start=True, stop=True,
                                )
                                nc.vector.tensor_copy(offS[:, nt, :],
                                                      zps[:, nt, 504:505])
                                nc.vector.tensor_copy(offB[:, nt, :],
                                                      zps[:, nt, 504:505])
                            for ct in range(CT):
                                for k in range(NK):
                                    nc.tensor.matmul(
                                        zps[:, ct, 502:503],
                                        KTs[:, k, ct * 128:(ct + 1) * 128],
                                        offB[:, k, :],
                                        start=(k == 0), stop=(k == NK - 1),
                                    )
                            # u = uL - K@offx  (per-partition scalar)
                            cS = wpool.tile([128, CT, 1], F32, tag="cs")
                            for ct in range(CT):
                                nc.vector.tensor_copy(cS[:, ct, :],
                                                      zps[:, ct, 502:503])
                                nc.vector.tensor_scalar_sub(
                                    ubf[:, ct, :], ubf[:, ct, :], cS[:, ct, :])
                            if last:
                                # X += offx (only needed for the final loss)
                                for nt in range(NK):
                                    nc.vector.tensor_scalar_add(
                                        Xbf[:, nt, :], Xbf[:, nt, :],
                                        offS[:, nt, :])
                            if debug and rep == 0 and it == 0:
                                u1f = wpool.tile([128, CT, TS], F32, tag="u1f")
                                for ct in range(CT):
                                    nc.vector.tensor_copy(u1f[:, ct, :],
                                                          ubf[:, ct, :])
                                nc.sync.dma_start(dbg_u1[:], u1f[:])

                    if debug and rep == 0:
                        uff = wpool.tile([128, CT, TS], F32, tag="uff")
                        for ct in range(CT):
                            nc.vector.tensor_copy(uff[:, ct, :], ubf[:, ct, :])
                        nc.sync.dma_start(dbg_uf[:], uff[:])
                        Xff = wpool.tile([128, NK, TS], F32, tag="xff")
                        for nt in range(NK):
                            nc.vector.tensor_copy(Xff[:, nt, :], Xbf[:, nt, :])
                        nc.sync.dma_start(dbg_X[:], Xff[:])

                    # ---- phase 5: losses = sum_n X*(QX) + sum_c u*(Ru) ----
                    prod = opool.tile([128, NK, TS], BF16)
                    prodr = opool.tile([128, CT, TS], BF16)
                    for ct in range(CT):
                        for k in range(CT):
                            nc.tensor.matmul(
                                zps[:, 4 + ct, 256:512],
                                Rs[:, k, ct * 128:(ct + 1) * 128],
                                ubf[:, k, :],
                                start=(k == 0), stop=(k == CT - 1),
                            )
                    for ct in range(CT):
                        nc.vector.tensor_tensor(prodr[:, ct, :], ubf[:, ct, :],
                                                zps[:, 4 + ct, 256:512], op=AL.mult)
                    for nt in range(NK):
                        for k in range(NK):
                            nc.tensor.matmul(
                                zps[:, nt, 0:TS],
                                Qs[:, k, nt * 128:(nt + 1) * 128],
                                Xbf[:, k, :],
                                start=(k == 0), stop=(k == NK - 1),
                            )
                    for nt in range(NK):
                        nc.vector.tensor_tensor(prod[:, nt, :], Xbf[:, nt, :],
                                                zps[:, nt, 0:TS], op=AL.mult)
                    for ct in range(CT):
                        nc.tensor.matmul(zps[0:1, 7, 256:256 + TS], ones[:],
                                         prodr[:, ct, :],
                                         start=(ct == 0), stop=False)
                    for nt in range(NK):
                        nc.tensor.matmul(zps[0:1, 7, 256:256 + TS], ones[:],
                                         prod[:, nt, :],
                                         start=False, stop=(nt == NK - 1))
                    loss = wpool.tile([1, TS], F32, tag="loss")
                    nc.vector.tensor_copy(loss[:], zps[0:1, 7, 256:256 + TS])
                    nc.sync.dma_start(loss_d[:], loss[:])
                    p5c.release()

    nc.compile()
    return nc


def _prep_inputs(inputs):
    f32 = np.float32
    bf = ml_dtypes.bfloat16
    E = np.asarray(inputs["E"], f32)            # [MC, N, M]
    K = np.asarray(inputs["K"], f32)            # [MC, N]
    E_stu = np.asarray(inputs["E_stu"], f32)    # [KF, MC, N]
    phi = np.asarray(inputs["phi"], f32)        # [T, KF]
    w = np.asarray(inputs["w_test"], f32)       # [T, N]
    Q = np.asarray(inputs["Q"], f32)
    R = np.asarray(inputs["R"], f32)
    bias = np.asarray(inputs["bias"], f32)

    ET = np.ascontiguousarray(E.transpose(2, 1, 0))          # [M, N, MC]
    Ecat = np.ascontiguousarray(E_stu.reshape(KF * MC, N)).astype(bf)
    KTb = np.ascontiguousarray(K.T).astype(bf)
    Qb = Q.astype(bf)
    Rb = R.astype(bf)
    biasT = np.ascontiguousarray(bias[:, None])
    # w^T padded with M-1 zero columns at the left (for t<0 history)
    wTp = np.concatenate([np.zeros((N, M - 1), f32), np.ascontiguousarray(w.T)], axis=1)
    phiT = np.ascontiguousarray(phi.T)                        # [KF, T]

    in_maps = []
    for r in range(NCORES):
        t0 = r * TS
        wT_r = np.ascontiguousarray(wTp[:, t0:t0 + TS + M - 1])
        phiB_r = np.broadcast_to(
            phiT[None, :, t0:t0 + TS], (128, KF, TS)
        ).astype(bf)
        mask_r = np.zeros((NCORES, 1), f32)
        mask_r[:r] = 1.0
        in_maps.append({
            "wT": wT_r, "ET": ET, "Ecat": Ecat, "KT": KTb,
            "Q": Qb, "R": Rb, "phiB": np.ascontiguousarray(phiB_r),
            "biasT": biasT, "mask": mask_r,
        })
    return in_maps


def kernel(**inputs) -> np.ndarray:
    if "nc" not in _CACHE:
        _CACHE["nc"] = build_nc()
    nc = _CACHE["nc"]
    in_maps = _prep_inputs(inputs)
    res = run_bass_kernel_spmd(nc, in_maps, list(range(NCORES)))
    out = np.concatenate([res.results[r]["loss"][0] for r in range(NCORES)])
    return out.astype(np.float32)


# revision 17
# speedup vs baseline: 2.8411x; 2.8411x over previous
"""Trainium2 Bass kernel for the GPCwSTU rollout (nn_GPCwSTU_72576357368005).

Math restructure (v2): the sequential rollout is the lower-triangular system
    u_t = d_t - K x_t,   x_t = Ecat^T p_t,   p_t = sum_{s<t} phi_s (x) u_s
with d_t = bias + sum_i E[:,:,i] @ w_{t-4+i} precomputable in parallel.
Richardson iteration in COMPOSED form (no Fmat precompute):
    u <- d - K @ scan_t(Ecat^T (phi (x) u))
Each iteration materializes X = scan(Ecat^T O) as a side product; after k
iterations, u = u^k and X = X(u^{k-1}) (one-iteration-stale X; fp64+bf16
mirror gives loss rel-err 1.7e-3 at k=2 vs the 2e-2 gate).

Time is sharded 256 steps/core across 8 cores; per iteration one AllGather of
per-core block sums of the z-steps provides the cross-core prefix offsets.
The scan runs offset-free (init 0) so the collective overlaps scan+stage2;
offsets are then folded in as per-partition scalars: u -= K@offx, X += offx.

Layouts are feature-major ([feature, t]). Phase 1 runs in float32r (1 PE
cycle/row at moving-dim 256 vs 4 for fp32).
"""

import os
import sys

sys.path.insert(0, "/opt/trn_rl_repo")

import numpy as np
import ml_dtypes

import concourse.bass as bass
import concourse.bacc as bacc
import concourse.mybir as mybir
from concourse import tile
from concourse.bass_utils import run_bass_kernel_spmd

BF16 = mybir.dt.bfloat16
F32 = mybir.dt.float32
F32R = mybir.dt.float32r
AL = mybir.AluOpType

T, N, MC, KF, M = 2048, 1024, 512, 20, 5
NCORES = 8
TS = T // NCORES          # 256 timesteps per core
NK = N // 128             # 8 tiles over state dim
CT = MC // 128            # 4 tiles over control dim
ICT = (KF * MC) // 128    # 80 tiles over the (filter, control) axis
NITERS = int(os.environ.get("K_NITERS", "2"))
SKIP_COLL = bool(int(os.environ.get("K_SKIP_COLL", "0")))  # timing expt only
GRP = 4                   # Ecat chunks per DMA group (1 MB transfers)
NGRP = ICT // GRP
BRES = int(os.environ.get("K_BRES", "8"))  # groups kept resident in SBUF
OBUF = 12                 # rotating O-chunk buffers

P1DT = F32R               # phase-1 matmul dtype (float32r: 4x faster than f32)

_CACHE = {}


def build_nc(debug=False, reps=1):
    nc = bacc.Bacc(None, target_bir_lowering=False, debug=False)

    # ---- I/O ----
    wT_d = nc.declare_dram_parameter("wT", [N, TS + M - 1], P1DT, isOutput=False)
    ET_d = nc.declare_dram_parameter("ET", [M, N, MC], P1DT, isOutput=False)
    Ecat_d = nc.declare_dram_parameter("Ecat", [KF * MC, N], BF16, isOutput=False)
    KT_d = nc.declare_dram_parameter("KT", [N, MC], BF16, isOutput=False)
    Q_d = nc.declare_dram_parameter("Q", [N, N], BF16, isOutput=False)
    R_d = nc.declare_dram_parameter("R", [MC, MC], BF16, isOutput=False)
    phiB_d = nc.declare_dram_parameter("phiB", [128, KF, TS], BF16, isOutput=False)
    biasT_d = nc.declare_dram_parameter("biasT", [MC, 1], F32, isOutput=False)
    mask_d = nc.declare_dram_parameter("mask", [NCORES, 1], F32, isOutput=False)
    loss_d = nc.declare_dram_parameter("loss", [1, TS], F32, isOutput=True)
    if debug:
        dbg_d = nc.declare_dram_parameter("dbg_d", [128, CT, TS], F32, isOutput=True)
        dbg_u1 = nc.declare_dram_parameter("dbg_u1", [128, CT, TS], F32, isOutput=True)
        dbg_uf = nc.declare_dram_parameter("dbg_uf", [128, CT, TS], F32, isOutput=True)
        dbg_X = nc.declare_dram_parameter("dbg_X", [128, NK, TS], F32, isOutput=True)

    # collective bounce buffers
    bxsum_d = nc.dram_tensor("bxsum", [N], F32)
    bxgat_d = nc.dram_tensor("bxgat", [NCORES, N], F32, addr_space="Shared")

    with tile.TileContext(nc) as tc:
        with (
            tc.tile_pool(name="const", bufs=1) as cpool,
            tc.tile_pool(name="live", bufs=1) as opool,
            tc.tile_pool(name="work", bufs=2) as wpool,
        ):
            # ---- small constants ----
            KTs = cpool.tile([128, NK, MC], BF16)
            nc.scalar.dma_start(KTs[:], KT_d.ap().rearrange("(k p) c -> p k c", p=128))
            phiB = cpool.tile([128, KF, TS], BF16)
            nc.scalar.dma_start(phiB[:], phiB_d[:])
            biasT = cpool.tile([128, CT, 1], F32)
            nc.scalar.dma_start(biasT[:], biasT_d.ap().rearrange("(c p) one -> p c one", p=128))
            mask = cpool.tile([NCORES, 1], F32)
            nc.scalar.dma_start(mask[:], mask_d[:])
            zeros = cpool.tile([128, TS], F32)
            nc.vector.memset(zeros[:], 0.0)
            ones = cpool.tile([128, 1], BF16)
            nc.vector.memset(ones[:], 1.0)
            gat0 = None
            if SKIP_COLL:
                gat0 = cpool.tile([NCORES, N], F32)
                nc.vector.memset(gat0[:], 0.0)

            # double-buffered d/ubf so the NEXT rep's phase 1 can run inside
            # the collective-wait window of the current rep
            dA = opool.tile([128, CT, TS], F32)
            dB = opool.tile([128, CT, TS], F32)
            ubfA = opool.tile([128, CT, TS], BF16)
            ubfB = opool.tile([128, CT, TS], BF16)
            p1 = tc.alloc_tile_pool(name="p1", bufs=1)
            ETr = ET_d.ap().rearrange("i (k p) c -> p i k c", p=128)

            def emit_phase1(zps, c0, dst_d, dst_ubf):
                wTs = p1.tile([128, NK, TS + M - 1], P1DT, tag="wts", name="wTs")
                nc.sync.dma_start(
                    wTs[:], wT_d.ap().rearrange("(k p) t -> p k t", p=128))
                for i in range(M):
                    ETs = p1.tile([128, NK, MC], P1DT, tag="ets", name="ETs")
                    eng = nc.scalar if i % 2 == 0 else nc.sync
                    eng.dma_start(ETs[:], ETr[:, i])
                    for ct in range(CT):
                        for k in range(NK):
                            nc.tensor.matmul(
                                zps[:, ct, c0:c0 + TS],
                                ETs[:, k, ct * 128:(ct + 1) * 128],
                                wTs[:, k, i:i + TS],
                                start=(i == 0 and k == 0),
                                stop=(i == M - 1 and k == NK - 1),
                            )
                for ct in range(CT):
                    nc.vector.tensor_scalar_add(
                        dst_d[:, ct, :], zps[:, ct, c0:c0 + TS], biasT[:, ct, :])
                    nc.vector.tensor_copy(dst_ubf[:, ct, :], dst_d[:, ct, :])

            for rep in range(reps):
                d = dA if rep % 2 == 0 else dB
                ubf = ubfA if rep % 2 == 0 else ubfB
                d_nxt = dB if rep % 2 == 0 else dA
                ubf_nxt = ubfB if rep % 2 == 0 else ubfA
                Xbf = opool.tile([128, NK, TS], BF16)

                with tc.tile_pool(name="ps", bufs=1, space="PSUM") as pspool:
                    # single PSUM tile: plane k <-> bank k
                    zps = pspool.tile([128, NK, 512], F32)

                    # ---- phase 1: d = bias + sum_i E_i @ w_shift_i ----
                    if rep == 0:
                        emit_phase1(zps, 0, d, ubf)
                        if debug:
                            nc.sync.dma_start(dbg_d[:], d[:])

                    # Q/R loads overlap the iterations; needed only in phase 5
                    p5c = tc.alloc_tile_pool(name="p5c", bufs=1)
                    Qs = p5c.tile([128, NK, N], BF16)
                    nc.gpsimd.dma_start(Qs[:], Q_d.ap().rearrange("(k p) n -> p k n", p=128))
                    Rs = p5c.tile([128, CT, MC], BF16)
                    nc.gpsimd.dma_start(Rs[:], R_d.ap().rearrange("(k p) c -> p k c", p=128))

                    # ---- Richardson iterations (composed form) ----
                    with (
                        tc.tile_pool(name="eres", bufs=1) as erpool,
                        tc.tile_pool(name="ecat", bufs=3) as epool,
                    ):
                        # first BRES groups stay resident after iteration 1
                        eres = erpool.tile([128, BRES * GRP, N], BF16)
                        for it in range(NITERS):
                            last = it == NITERS - 1
                            # stage 1: zsteps = Ecat^T @ (phi (x) u), with the
                            # O chunks built just-in-time from a rotating pool
                            for g in range(NGRP):
                                if g < BRES:
                                    eg = eres[:, g * GRP:(g + 1) * GRP, :]
                                    if it == 0:
                                        eng = nc.sync if g % 2 == 0 else nc.scalar
                                        eng.dma_start(
                                            eg,
                                            Ecat_d[g * GRP * 128:(g + 1) * GRP * 128, :]
                                            .rearrange("(a p) n -> p a n", p=128))
                                else:
                                    eg = epool.tile([128, GRP, N], BF16, tag="eg")
                                    eng = nc.sync if g % 2 == 0 else nc.scalar
                                    eng.dma_start(
                                        eg[:],
                                        Ecat_d[g * GRP * 128:(g + 1) * GRP * 128, :]
                                        .rearrange("(a p) n -> p a n", p=128))
                                for a in range(GRP):
                                    kk = g * GRP + a
                                    Oc = opool.tile([128, TS], BF16, tag="oc",
                                                    bufs=OBUF, name="Oc")
                                    nc.vector.tensor_tensor(
                                        Oc[:], ubf[:, kk % CT, :],
                                        phiB[:, kk // CT, :], op=AL.mult)
                                    for nt in range(NK):
                                        nc.tensor.matmul(
                                            zps[:, nt, 0:TS],
                                            eg[:, a, nt * 128:(nt + 1) * 128],
                                            Oc[:],
                                            start=(kk == 0), stop=(kk == ICT - 1),
                                        )
                            # block sums -> AllGather (overlaps scan+stage2)
                            Bz = wpool.tile([128, NK, 1], F32, tag="bz")
                            for nt in range(NK):
                                nc.vector.reduce_sum(Bz[:, nt, :], zps[:, nt, 0:TS],
                                                     axis=mybir.AxisListType.X)
                                nc.gpsimd.dma_start(
                                    bxsum_d[nt * 128:(nt + 1) * 128], Bz[:, nt, :])
                            if not SKIP_COLL:
                                nc.gpsimd.collective_compute(
                                    "AllGather", AL.bypass,
                                    ins=[bxsum_d[:]], outs=[bxgat_d[:]],
                                    replica_groups=[list(range(NCORES))],
                                )
                                gatx = wpool.tile([NCORES, N], F32, tag="gatx")
                                nc.gpsimd.dma_start(gatx[:], bxgat_d[:])
                            else:
                                gatx = gat0
                            # X = exclusive-scan(zsteps), offset-free
                            for nt in range(NK):
                                nc.vector.tensor_copy(Xbf[:, nt, 0:1], zeros[:, 0:1])
                                nc.vector.tensor_tensor_scan(
                                    Xbf[:, nt, 1:TS], zps[:, nt, 0:TS - 1],
                                    zeros[:, 0:TS - 1], 0.0,
                                    op0=AL.add, op1=AL.add)
                            # stage 2: uL = d - K @ Xlocal
                            for ct in range(CT):
                                for k in range(NK):
                                    nc.tensor.matmul(
                                        zps[:, 4 + ct, 256:512],
                                        KTs[:, k, ct * 128:(ct + 1) * 128],
                                        Xbf[:, k, :],
                                        start=(k == 0), stop=(k == NK - 1),
                                    )
                            for ct in range(CT):
                                nc.vector.tensor_sub(
                                    ubf[:, ct, :], d[:, ct, :],
                                    zps[:, 4 + ct, 256:512])
                            # pipeline the NEXT rep's phase 1 into the
                            # collective-wait window (banks 0-3, cols 256:512)
                            if it == 0 and rep + 1 < reps:
                                emit_phase1(zps, 256, d_nxt, ubf_nxt)
                            # offsets: offx = sum_{r'<r} Bz_{r'};  c = K @ offx
                            # (offp/cS live in banks 4-7, dead stage-2 region)
                            offS = wpool.tile([128, NK, 1], F32, tag="offs")
                            offB = wpool.tile([128, NK, 1], BF16, tag="offb")
                            for nt in range(NK):
                                oslc = zps[:, 4 + nt % 4,
                                           496 + nt // 4:497 + nt // 4]
                                nc.tensor.matmul(
                                    oslc,
                                    gatx[:, nt * 128:(nt + 1) * 128], mask[:],
                                    start=True, stop=True,
                                )
                                nc.vector.tensor_copy(offS[:, nt, :], oslc)
                                nc.vector.tensor_copy(offB[:, nt, :], oslc)
                            for ct in range(CT):
                                for k in range(NK):
                                    nc.tensor.matmul(
                                        zps[:, 4 + ct, 494:495],
                                        KTs[:, k, ct * 128:(ct + 1) * 128],
                                        offB[:, k, :],
                                        start=(k == 0), stop=(k == NK - 1),
                                    )
                            # u = uL - K@offx  (per-partition scalar)
                            cS = wpool.tile([128, CT, 1], F32, tag="cs")
                            for ct in range(CT):
                                nc.vector.tensor_copy(cS[:, ct, :],
                                                      zps[:, 4 + ct, 494:495])
                                nc.vector.tensor_scalar_sub(
                                    ubf[:, ct, :], ubf[:, ct, :], cS[:, ct, :])
                            if last:
                                # X += offx (only needed for the final loss)
                                for nt in range(NK):
                                    nc.vector.tensor_scalar_add(
                                        Xbf[:, nt, :], Xbf[:, nt, :],
                                        offS[:, nt, :])
                            if debug and rep == 0 and it == 0:
                                u1f = wpool.tile([128, CT, TS], F32, tag="u1f")
                                for ct in range(CT):
                                    nc.vector.tensor_copy(u1f[:, ct, :],
                                                          ubf[:, ct, :])
                                nc.sync.dma_start(dbg_u1[:], u1f[:])

                    if debug and rep == 0:
                        uff = wpool.tile([128, CT, TS], F32, tag="uff")
                        for ct in range(CT):
                            nc.vector.tensor_copy(uff[:, ct, :], ubf[:, ct, :])
                        nc.sync.dma_start(dbg_uf[:], uff[:])
                        Xff = wpool.tile([128, NK, TS], F32, tag="xff")
                        for nt in range(NK):
                            nc.vector.tensor_copy(Xff[:, nt, :], Xbf[:, nt, :])
                        nc.sync.dma_start(dbg_X[:], Xff[:])

                    # ---- phase 5: losses = sum_n X*(QX) + sum_c u*(Ru) ----
                    prod = opool.tile([128, NK, TS], BF16)
                    prodr = opool.tile([128, CT, TS], BF16)
                    for ct in range(CT):
                        for k in range(CT):
                            nc.tensor.matmul(
                                zps[:, 4 + ct, 256:512],
                                Rs[:, k, ct * 128:(ct + 1) * 128],
                                ubf[:, k, :],
                                start=(k == 0), stop=(k == CT - 1),
                            )
                    for ct in range(CT):
                        nc.vector.tensor_tensor(prodr[:, ct, :], ubf[:, ct, :],
                                                zps[:, 4 + ct, 256:512], op=AL.mult)
                    for nt in range(NK):
                        for k in range(NK):
                            nc.tensor.matmul(
                                zps[:, nt, 0:TS],
                                Qs[:, k, nt * 128:(nt + 1) * 128],
                                Xbf[:, k, :],
                                start=(k == 0), stop=(k == NK - 1),
                            )
                    for nt in range(NK):
                        nc.vector.tensor_tensor(prod[:, nt, :], Xbf[:, nt, :],
                                                zps[:, nt, 0:TS], op=AL.mult)
                    for ct in range(CT):
                        nc.tensor.matmul(zps[0:1, 7, 256:256 + TS], ones[:],
                                         prodr[:, ct, :],
                                         start=(ct == 0), stop=False)
                    for nt in range(NK):
                        nc.tensor.matmul(zps[0:1, 7, 256:256 + TS], ones[:],
                                         prod[:, nt, :],
                                         start=False, stop=(nt == NK - 1))
                    loss = wpool.tile([1, TS], F32, tag="loss")
                    nc.vector.tensor_copy(loss[:], zps[0:1, 7, 256:256 + TS])
                    nc.sync.dma_start(loss_d[:], loss[:])
                    p5c.release()
            p1.release()

    nc.compile()
    return nc


def _prep_inputs(inputs):
    f32 = np.float32
    bf = ml_dtypes.bfloat16
    E = np.asarray(inputs["E"], f32)            # [MC, N, M]
    K = np.asarray(inputs["K"], f32)            # [MC, N]
    E_stu = np.asarray(inputs["E_stu"], f32)    # [KF, MC, N]
    phi = np.asarray(inputs["phi"], f32)        # [T, KF]
    w = np.asarray(inputs["w_test"], f32)       # [T, N]
    Q = np.asarray(inputs["Q"], f32)
    R = np.asarray(inputs["R"], f32)
    bias = np.asarray(inputs["bias"], f32)

    ET = np.ascontiguousarray(E.transpose(2, 1, 0))          # [M, N, MC]
    Ecat = np.ascontiguousarray(E_stu.reshape(KF * MC, N)).astype(bf)
    KTb = np.ascontiguousarray(K.T).astype(bf)
    Qb = Q.astype(bf)
    Rb = R.astype(bf)
    biasT = np.ascontiguousarray(bias[:, None])
    # w^T padded with M-1 zero columns at the left (for t<0 history)
    wTp = np.concatenate([np.zeros((N, M - 1), f32), np.ascontiguousarray(w.T)], axis=1)
    phiT = np.ascontiguousarray(phi.T)                        # [KF, T]

    in_maps = []
    for r in range(NCORES):
        t0 = r * TS
        wT_r = np.ascontiguousarray(wTp[:, t0:t0 + TS + M - 1])
        phiB_r = np.broadcast_to(
            phiT[None, :, t0:t0 + TS], (128, KF, TS)
        ).astype(bf)
        mask_r = np.zeros((NCORES, 1), f32)
        mask_r[:r] = 1.0
        in_maps.append({
            "wT": wT_r, "ET": ET, "Ecat": Ecat, "KT": KTb,
            "Q": Qb, "R": Rb, "phiB": np.ascontiguousarray(phiB_r),
            "biasT": biasT, "mask": mask_r,
        })
    return in_maps


def kernel(**inputs) -> np.ndarray:
    if "nc" not in _CACHE:
        _CACHE["nc"] = build_nc()
    nc = _CACHE["nc"]
    in_maps = _prep_inputs(inputs)
    res = run_bass_kernel_spmd(nc, in_maps, list(range(NCORES)))
    out = np.concatenate([res.results[r]["loss"][0] for r in range(NCORES)])
    return out.astype(np.float32)
